# revision 2
# baseline (speedup 1.0000x reference)
"""Trainium2 Bass kernel for per-batch channel attention (CxAM-style).

Reference (per batch element b):
    q = (Wq @ x_b + bq)        # [64, T]
    k = (Wk @ x_b + bk)        # [64, T]
    v = (Wv @ x_b + bv)        # [512, T]
    R = q.T @ k                # [T, T]
    A = softmax(R, axis=-1)
    out_b = v @ A.T            # [512, T]

Sharding: pure data-parallel — batch B=8, one batch element per NeuronCore.

Per-core algorithm (layouts chosen so no attention-matrix transposes are
needed and every heavy matmul has free dim 512 in bf16 => full PE rate):
    QK   [128, T] bf16   rows 0:64 = Q, 64:128 = K  (packed projection)
    VT   [s=128 x 16, c=512] bf16 = x.T @ Wv.T + bv (V transposed, bias in)
    per t-block of 512, per s-chunk pair (row-packed on the PE array):
      ST_j [s=128, t=512] = K_chunk.T @ Q_block      (scores, transposed)
      E_j = exp(ST_j)  (bf16; no max needed: |R| <= ~11)
      U_ck [c=128, t] += VT_chunk_ck.T @ E_j         (unnormalized out)
    The softmax denominator is computed OFF the PE: a DVE binary tree sums
    the 16 E_j tiles elementwise ([128, t] partial sums), then a GPSIMD
    partition_all_reduce folds the 128 partitions (result broadcast to all
    partitions, which doubles as the normalizer broadcast), then a DVE
    fast reciprocal gives rb[128, t] and out = U * rb.
The s-chunk loop is software-pipelined one pair deep so the exp latency
(ACT) hides under the previous pair's consume matmuls (PE).
"""

import os

os.environ.setdefault("MYCRO_LOCAL_CACHE", "1")

import numpy as np

import concourse.bass as bass
import concourse.mybir as mybir
import concourse.tile as tile
from concourse import bacc
from concourse import bass_isa
from concourse.bass_utils import run_bass_kernel_spmd
from concourse.masks import make_identity

F32 = mybir.dt.float32
F32R = mybir.dt.float32r
BF16 = mybir.dt.bfloat16
AF = mybir.ActivationFunctionType

B = 8
C = 512
T = 2048
CQ = 64
NCORES = 8

TB = 512            # t-block (free dim of main matmuls)
NTB = T // TB       # 4
NSC = T // 128      # 16 s-chunks
NPAIR = NSC // 2    # 8 row-packed score pairs per t-block
NCH = C // 128      # 4 contraction chunks
NCC = C // 128      # 4 output channel chunks


def _build_program() -> bass.Bass:
    nc = bacc.Bacc("TRN2", target_bir_lowering=False, debug=False, num_devices=NCORES)

    x_d = nc.declare_dram_parameter("x", [C, T], F32, isOutput=False)
    wq_d = nc.declare_dram_parameter("Wq", [CQ, C], F32, isOutput=False)
    bq_d = nc.declare_dram_parameter("bq", [CQ, 1], F32, isOutput=False)
    wk_d = nc.declare_dram_parameter("Wk", [CQ, C], F32, isOutput=False)
    bk_d = nc.declare_dram_parameter("bk", [CQ, 1], F32, isOutput=False)
    wv_d = nc.declare_dram_parameter("Wv", [C, C], F32, isOutput=False)
    bv_d = nc.declare_dram_parameter("bv", [1, C], F32, isOutput=False)
    out_d = nc.declare_dram_parameter("out", [C, T], F32, isOutput=True)

    with tile.TileContext(nc) as tc:
        with (
            tc.tile_pool(name="const", bufs=1) as const,
            tc.tile_pool(name="weights", bufs=1) as wpool,
        ):
            ident = const.tile([128, 128], F32)
            make_identity(nc, ident[:])
            ones_row = const.tile([1, 128], F32)
            nc.gpsimd.memset(ones_row[:], 1.0)

            # ---- raw inputs -> SBUF
            wq_s = wpool.tile([CQ, C], F32)
            nc.sync.dma_start(out=wq_s[:], in_=wq_d[:])
            wk_s = wpool.tile([CQ, C], F32)
            nc.sync.dma_start(out=wk_s[:], in_=wk_d[:])
            wv_s = wpool.tile([128, NCH, C], F32)
            nc.sync.dma_start(
                out=wv_s[:], in_=wv_d[:].rearrange("(po pi) c -> pi po c", pi=128)
            )
            bqk = wpool.tile([128, 1], F32)
            nc.sync.dma_start(out=bqk[0:CQ, :], in_=bq_d[:])
            nc.sync.dma_start(out=bqk[CQ:128, :], in_=bk_d[:])
            bv_row = wpool.tile([1, C], F32)
            nc.sync.dma_start(out=bv_row[:], in_=bv_d[:])
            # x arrives per channel-chunk (contiguous 8 KB per partition) so
            # casts and partial projections pipeline with the DMA
            x_s = wpool.tile([128, NCH, T], F32)
            x_bf = wpool.tile([128, NCH, T], BF16)
            x_r = x_d[:].rearrange("(po pi) t -> pi po t", pi=128)
            for ci in range(NCH):
                nc.sync.dma_start(out=x_s[:, ci, :], in_=x_r[:, ci, :])
                for th in range(2):
                    ths = slice(th * T // 2, (th + 1) * T // 2)
                    if th == 0:
                        nc.vector.tensor_copy(x_bf[:, ci, ths], x_s[:, ci, ths])
                    else:
                        nc.scalar.activation(x_bf[:, ci, ths], x_s[:, ci, ths], AF.Copy)

            # ---- transpose weights on PE
            wqkT = wpool.tile([128, NCH, 128], BF16)  # [ch, chunk, 0:64 WqT | 64:128 WkT]
            wvT = wpool.tile([128, NCH, C], BF16)     # [ch, chunk, c]
            with tc.tile_pool(name="psum_w", bufs=4, space="PSUM") as psum_w:
                for j in range(NCH):
                    ptq = psum_w.tile([128, CQ], F32, tag="pt")
                    nc.tensor.transpose(
                        ptq[:], wq_s[:, j * 128:(j + 1) * 128], ident[0:CQ, 0:CQ]
                    )
                    nc.vector.tensor_copy(wqkT[:, j, 0:CQ], ptq[:])
                    ptk = psum_w.tile([128, CQ], F32, tag="pt")
                    nc.tensor.transpose(
                        ptk[:], wk_s[:, j * 128:(j + 1) * 128], ident[0:CQ, 0:CQ]
                    )
                    nc.vector.tensor_copy(wqkT[:, j, CQ:128], ptk[:])
                for i in range(NCH):       # c chunk of Wv rows
                    for j in range(NCH):   # ch chunk of Wv cols
                        ptv = psum_w.tile([128, 128], F32, tag="pt")
                        nc.tensor.transpose(
                            ptv[:], wv_s[:, i, j * 128:(j + 1) * 128], ident[:]
                        )
                        nc.vector.tensor_copy(
                            wvT[:, j, i * 128:(i + 1) * 128], ptv[:]
                        )

            qk = wpool.tile([128, T], BF16)   # rows 0:64 Q, 64:128 K
            kq = wpool.tile([128, T], BF16)   # rows 0:64 K, 64:128 Q
            vT = wpool.tile([128, NSC, C], BF16)
            bv_bcast = wpool.tile([128, C], F32)

            with tc.tile_pool(name="psum_p", bufs=1, space="PSUM") as psum_p:
                # bv broadcast [1, C] -> [128, C]
                bvb = psum_p.tile([128, C], F32, tag="bvb", bufs=1)
                nc.tensor.matmul(
                    bvb[:], ones_row[:], bv_row[:], start=True, stop=True
                )
                nc.vector.tensor_copy(bv_bcast[:], bvb[:])

                # projections, interleaved per t-chunk so they start as soon
                # as that x chunk has landed
                for tt in range(NTB):
                    # packed Q/K projection: out rows 0:64 = Q, 64:128 = K
                    ps = psum_p.tile(
                        [128, TB], F32, tag="qkproj", bufs=3, name=f"qkp_{tt}"
                    )
                    for ci in range(NCH):
                        nc.tensor.matmul(
                            ps[:],
                            wqkT[:, ci, :],
                            x_bf[:, ci, tt * TB:(tt + 1) * TB],
                            start=(ci == 0),
                            stop=(ci == NCH - 1),
                        )
                    nc.vector.tensor_scalar_add(
                        qk[:, tt * TB:(tt + 1) * TB], ps[:], bqk[:, 0:1]
                    )

                    # V^T projection: vT[s, c] = x.T @ Wv.T + bv
                    for j in range(4 * tt, 4 * tt + 4):
                        psv = psum_p.tile(
                            [128, C], F32, tag="vproj", bufs=4, name=f"vp_{j}"
                        )
                        for ci in range(NCH):
                            nc.tensor.matmul(
                                psv[:],
                                x_bf[:, ci, j * 128:(j + 1) * 128],
                                wvT[:, ci, :],
                                start=(ci == 0),
                                stop=(ci == NCH - 1),
                            )
                        nc.vector.tensor_add(vT[:, j, :], psv[:], bv_bcast[:])

            # swap-duplicate for row-packed score matmuls
            nc.sync.dma_start(out=kq[0:CQ, :], in_=qk[CQ:128, :])
            nc.sync.dma_start(out=kq[CQ:128, :], in_=qk[0:CQ, :])

            # ---- main attention loop, software-pipelined one pair deep
            with (
                tc.tile_pool(name="et", bufs=2) as et_pool,
                tc.tile_pool(name="ps_sc", bufs=2, space="PSUM") as ps_sc,
                tc.tile_pool(name="ps_av", bufs=1, space="PSUM") as ps_av,
                tc.tile_pool(name="small", bufs=2) as small,
                tc.tile_pool(name="outp", bufs=2) as outp,
            ):
                avs = {}
                ets = {}

                def start_block(tb):
                    avs[tb] = [
                        ps_av.tile([128, TB], F32, tag=f"av{ck}", name=f"av{ck}_{tb}")
                        for ck in range(NCC)
                    ]
                    ets[tb] = et_pool.tile(
                        [128, NSC, TB], BF16, tag="et", name=f"et_{tb}"
                    )

                def emit_scores(tb, jp):
                    tsl = slice(tb * TB, (tb + 1) * TB)
                    j0, j1 = 2 * jp, 2 * jp + 1
                    sc = ps_sc.tile(
                        [128, 2, TB], F32, tag="sc", name=f"sc_{tb}_{jp}"
                    )
                    nc.tensor.matmul(
                        sc[:, 0, :],
                        kq[0:CQ, j0 * 128:(j0 + 1) * 128],
                        qk[0:CQ, tsl],
                        start=True,
                        stop=True,
                    )
                    nc.tensor.matmul(
                        sc[:, 1, :],
                        qk[CQ:128, j1 * 128:(j1 + 1) * 128],
                        kq[CQ:128, tsl],
                        start=True,
                        stop=True,
                        tile_position=(64, 0),
                    )
                    # one batched exp over both score tiles (2 PSUM banks)
                    nc.scalar.activation(
                        ets[tb][:, j0:j0 + 2, :], sc[:, :, :], AF.Exp
                    )

                def emit_consume(tb, jp):
                    for idx in (0, 1):
                        j = 2 * jp + idx
                        for ck in range(NCC):
                            nc.tensor.matmul(
                                avs[tb][ck][:],
                                vT[:, j, ck * 128:(ck + 1) * 128],
                                ets[tb][:, j, :],
                                start=(j == 0),
                                stop=(j == NSC - 1),
                            )

                def finish_block(tb):
                    tsl = slice(tb * TB, (tb + 1) * TB)
                    et = ets[tb]
                    # DVE tree-sum of the 16 E tiles -> esum [128, TB]
                    e8 = small.tile([128, 8, TB], BF16, tag="e8", name=f"e8_{tb}")
                    nc.vector.tensor_add(e8[:], et[:, 0:8, :], et[:, 8:16, :])
                    e4 = small.tile([128, 4, TB], BF16, tag="e4", name=f"e4_{tb}")
                    nc.vector.tensor_add(e4[:], e8[:, 0:4, :], e8[:, 4:8, :])
                    e2 = small.tile([128, 2, TB], BF16, tag="e2", name=f"e2_{tb}")
                    nc.vector.tensor_add(e2[:], e4[:, 0:2, :], e4[:, 2:4, :])
                    esum = small.tile([128, TB], F32, tag="esum", name=f"esum_{tb}")
                    nc.vector.tensor_add(esum[:], e2[:, 0, :], e2[:, 1, :])
                    # fold partitions on GPSIMD; result broadcast to all rows
                    dsum = small.tile([128, TB], F32, tag="dsum", name=f"dsum_{tb}")
                    nc.gpsimd.partition_all_reduce(
                        dsum[:], esum[:], channels=128,
                        reduce_op=bass_isa.ReduceOp.add,
                    )
                    rb = small.tile([128, TB], F32, tag="rb", name=f"rb_{tb}")
                    nc.vector.reciprocal_approx_fast(rb[:], dsum[:])

                    for ck in range(NCC):
                        ot = outp.tile(
                            [128, TB], F32, tag=f"ot{ck}", name=f"ot{ck}_{tb}"
                        )
                        nc.vector.tensor_mul(ot[:], avs[tb][ck][:], rb[:])
                        nc.sync.dma_start(
                            out=out_d[ck * 128:(ck + 1) * 128, tsl], in_=ot[:]
                        )

                pending = None  # (tb, jp)
                for tb in range(NTB):
                    start_block(tb)
                    for jp in range(NPAIR):
                        emit_scores(tb, jp)
                        if pending is not None:
                            ptb, pjp = pending
                            emit_consume(ptb, pjp)
                            if pjp == NPAIR - 1:
                                finish_block(ptb)
                        pending = (tb, jp)
                ptb, pjp = pending
                emit_consume(ptb, pjp)
                finish_block(ptb)

    nc.compile()
    return nc


_PROGRAM = None


def _get_program() -> bass.Bass:
    global _PROGRAM
    if _PROGRAM is None:
        _PROGRAM = _build_program()
    return _PROGRAM


def kernel(**inputs: np.ndarray) -> np.ndarray:
    x = np.ascontiguousarray(np.asarray(inputs["x"], dtype=np.float32))
    wq = np.ascontiguousarray(np.asarray(inputs["Wq"], dtype=np.float32))
    bq = np.ascontiguousarray(np.asarray(inputs["bq"], dtype=np.float32)).reshape(CQ, 1)
    wk = np.ascontiguousarray(np.asarray(inputs["Wk"], dtype=np.float32))
    bk = np.ascontiguousarray(np.asarray(inputs["bk"], dtype=np.float32)).reshape(CQ, 1)
    wv = np.ascontiguousarray(np.asarray(inputs["Wv"], dtype=np.float32))
    bv = np.ascontiguousarray(np.asarray(inputs["bv"], dtype=np.float32)).reshape(1, C)

    nc = _get_program()
    in_maps = [
        {
            "x": np.ascontiguousarray(x[b]),
            "Wq": wq,
            "bq": bq,
            "Wk": wk,
            "bk": bk,
            "Wv": wv,
            "bv": bv,
        }
        for b in range(NCORES)
    ]
    res = run_bass_kernel_spmd(nc, in_maps, list(range(NCORES)))
    out = np.stack([res.results[b]["out"] for b in range(NCORES)], axis=0)
    return out.astype(np.float32)


if __name__ == "__main__":
    import reference

    inputs = {k: np.asarray(v) for k, v in reference.setup_inputs().items()}
    expected = np.asarray(reference.reference(**inputs))
    actual = kernel(**inputs)
    rel = np.linalg.norm(actual - expected) / np.linalg.norm(expected)
    print("Relative error:", rel)


# revision 4
# speedup vs baseline: 1.1149x; 1.1149x over previous
"""Trainium2 Bass kernel for per-batch channel attention (CxAM-style).

Reference (per batch element b):
    q = (Wq @ x_b + bq)        # [64, T]
    k = (Wk @ x_b + bk)        # [64, T]
    v = (Wv @ x_b + bv)        # [512, T]
    R = q.T @ k                # [T, T]
    A = softmax(R, axis=-1)
    out_b = v @ A.T            # [512, T]

Sharding: pure data-parallel — batch B=8, one batch element per NeuronCore.

Per-core algorithm:
    x is DMA'd in four t-chunks (all channels per chunk) so the Q/K/V
    projections and even t-block 0's score/AV pipeline chase the DMA.
    Both packings of the Q/K projection are computed ([q|k] and [k|q])
    so the row-packed score matmuls need no SBUF duplication DMA.
    Per t-block of 512, per s-chunk pair (row-packed on the PE array):
      ST_j [s=128, t=512] = K_chunk.T @ Q_block      (scores, transposed)
      E_j = exp(ST_j)   one batched ACT op per pair  (bf16)
      U_ck [c=128, t] += VT_chunk_ck.T @ E_j         (unnormalized out)
    The softmax denominator never touches the PE: a running DVE
    accumulation sums the 16 E_j tiles elementwise, a GPSIMD
    partition_all_reduce folds the 128 partitions (result broadcast to
    all partitions = the normalizer broadcast), and a DVE fast
    reciprocal yields rb with out = U * rb.
    The U accumulation is split into halves A=(ck0,ck1)/B=(ck2,ck3);
    B of block tb runs during block tb+1.  This needs only 2+2 PSUM
    banks (leaving 4 for double-buffered score pairs) and hides the
    reciprocal-chain latency entirely.
"""

import os

os.environ.setdefault("MYCRO_LOCAL_CACHE", "1")

import numpy as np

import concourse.bass as bass
import concourse.mybir as mybir
import concourse.tile as tile
from concourse import bacc
from concourse import bass_isa
from concourse.bass_utils import run_bass_kernel_spmd
from concourse.masks import make_identity

F32 = mybir.dt.float32
BF16 = mybir.dt.bfloat16
AF = mybir.ActivationFunctionType

B = 8
C = 512
T = 2048
CQ = 64
NCORES = 8

TB = 512            # t-block (free dim of main matmuls)
NTB = T // TB       # 4
NSC = T // 128      # 16 s-chunks
NPAIR = NSC // 2    # 8 row-packed score pairs per t-block
NCH = C // 128      # 4 contraction chunks
NCC = C // 128      # 4 output channel chunks


def _build_program() -> bass.Bass:
    nc = bacc.Bacc("TRN2", target_bir_lowering=False, debug=False, num_devices=NCORES)

    x_d = nc.declare_dram_parameter("x", [C, T], F32, isOutput=False)
    wq_d = nc.declare_dram_parameter("Wq", [CQ, C], F32, isOutput=False)
    bq_d = nc.declare_dram_parameter("bq", [CQ, 1], F32, isOutput=False)
    wk_d = nc.declare_dram_parameter("Wk", [CQ, C], F32, isOutput=False)
    bk_d = nc.declare_dram_parameter("bk", [CQ, 1], F32, isOutput=False)
    wv_d = nc.declare_dram_parameter("Wv", [C, C], F32, isOutput=False)
    bv_d = nc.declare_dram_parameter("bv", [1, C], F32, isOutput=False)
    out_d = nc.declare_dram_parameter("out", [C, T], F32, isOutput=True)

    with tile.TileContext(nc) as tc:
        with (
            tc.tile_pool(name="const", bufs=1) as const,
            tc.tile_pool(name="weights", bufs=1) as wpool,
        ):
            ident_bf = const.tile([128, 128], BF16)
            make_identity(nc, ident_bf[:])
            ones_row = const.tile([1, 128], F32)
            nc.gpsimd.memset(ones_row[:], 1.0)

            # ---- weight DMAs first (wv is the big one; x chunks follow)
            wv_s = wpool.tile([128, NCH, C], F32)
            nc.sync.dma_start(
                out=wv_s[:], in_=wv_d[:].rearrange("(po pi) c -> pi po c", pi=128)
            )
            wq_s = wpool.tile([CQ, C], F32)
            nc.sync.dma_start(out=wq_s[:], in_=wq_d[:])
            wk_s = wpool.tile([CQ, C], F32)
            nc.sync.dma_start(out=wk_s[:], in_=wk_d[:])
            bqk = wpool.tile([128, 1], F32)    # [bq; bk]
            bkq = wpool.tile([128, 1], F32)    # [bk; bq]
            nc.sync.dma_start(out=bqk[0:CQ, :], in_=bq_d[:])
            nc.sync.dma_start(out=bqk[CQ:128, :], in_=bk_d[:])
            nc.sync.dma_start(out=bkq[0:CQ, :], in_=bk_d[:])
            nc.sync.dma_start(out=bkq[CQ:128, :], in_=bq_d[:])
            bv_row = wpool.tile([1, C], F32)
            nc.sync.dma_start(out=bv_row[:], in_=bv_d[:])

            # x arrives in four t-chunks carrying all channels, so per-chunk
            # projections (and t-block 0's pipeline) chase the DMA
            x_s = wpool.tile([128, NCH, T], F32)
            x_bf = wpool.tile([128, NCH, T], BF16)
            x_r = x_d[:].rearrange("(po pi) t -> pi po t", pi=128)

            # ---- cast weights to bf16, transpose on PE (single-pass bf16)
            wv_bf = wpool.tile([128, NCH, C], BF16)
            nc.vector.tensor_copy(wv_bf[:], wv_s[:])
            wq_bf = wpool.tile([CQ, C], BF16)
            nc.vector.tensor_copy(wq_bf[:], wq_s[:])
            wk_bf = wpool.tile([CQ, C], BF16)
            nc.vector.tensor_copy(wk_bf[:], wk_s[:])

            wqkT = wpool.tile([128, NCH, 128], BF16)  # [c, chunk, 0:64 WqT | 64:128 WkT]
            wkqT = wpool.tile([128, NCH, 128], BF16)  # [c, chunk, 0:64 WkT | 64:128 WqT]
            wvT = wpool.tile([128, NCH, C], BF16)     # [c, chunk, cout]
            with tc.tile_pool(name="psum_w", bufs=4, space="PSUM") as psum_w:
                for i in range(NCH):       # c chunk of Wv rows
                    for j in range(NCH):   # ch chunk of Wv cols
                        ptv = psum_w.tile([128, 128], BF16, tag="pt")
                        nc.tensor.transpose(
                            ptv[:], wv_bf[:, i, j * 128:(j + 1) * 128], ident_bf[:]
                        )
                        nc.vector.tensor_copy(
                            wvT[:, j, i * 128:(i + 1) * 128], ptv[:]
                        )
                for j in range(NCH):
                    ptq = psum_w.tile([128, CQ], BF16, tag="ptq")
                    nc.tensor.transpose(
                        ptq[:], wq_bf[:, j * 128:(j + 1) * 128], ident_bf[0:CQ, 0:CQ]
                    )
                    nc.vector.tensor_copy(wqkT[:, j, 0:CQ], ptq[:])
                    nc.vector.tensor_copy(wkqT[:, j, CQ:128], ptq[:])
                    ptk = psum_w.tile([128, CQ], BF16, tag="ptq")
                    nc.tensor.transpose(
                        ptk[:], wk_bf[:, j * 128:(j + 1) * 128], ident_bf[0:CQ, 0:CQ]
                    )
                    nc.vector.tensor_copy(wqkT[:, j, CQ:128], ptk[:])
                    nc.vector.tensor_copy(wkqT[:, j, 0:CQ], ptk[:])

            qk = wpool.tile([128, T], BF16)   # rows 0:64 Q, 64:128 K
            kq = wpool.tile([128, T], BF16)   # rows 0:64 K, 64:128 Q
            vT = wpool.tile([128, NSC, C], BF16)
            bv_bcast = wpool.tile([128, C], F32)

            # ---- main PSUM pool: 4 score banks + 2+2 AV banks
            with (
                tc.tile_pool(name="et", bufs=2) as et_pool,
                tc.tile_pool(name="ps", bufs=1, space="PSUM") as ps,
                tc.tile_pool(name="small", bufs=2) as small,
                tc.tile_pool(name="outp", bufs=2) as outp,
            ):
                avA = {}
                avB = {}
                ets = {}
                esums = {}
                rbs = {}

                # bv broadcast [1, C] -> [128, C] (borrows an AV bank)
                bvb = ps.tile([128, C], F32, tag="ava0", name="bvb")
                nc.tensor.matmul(
                    bvb[:], ones_row[:], bv_row[:], start=True, stop=True
                )
                nc.vector.tensor_copy(bv_bcast[:], bvb[:])

                def start_block(tb):
                    avA[tb] = [
                        ps.tile([128, TB], F32, tag=f"ava{ck}", name=f"avA{ck}_{tb}")
                        for ck in range(2)
                    ]
                    ets[tb] = et_pool.tile(
                        [128, NSC, TB], BF16, tag="et", name=f"et_{tb}"
                    )

                def emit_scores(tb, jp):
                    """Scores + batched exp + running denominator accumulation."""
                    tsl = slice(tb * TB, (tb + 1) * TB)
                    j0, j1 = 2 * jp, 2 * jp + 1
                    sc = ps.tile(
                        [128, 2, TB], F32, tag="sc", bufs=2, name=f"sc_{tb}_{jp}"
                    )
                    nc.tensor.matmul(
                        sc[:, 0, :],
                        kq[0:CQ, j0 * 128:(j0 + 1) * 128],
                        qk[0:CQ, tsl],
                        start=True,
                        stop=True,
                    )
                    nc.tensor.matmul(
                        sc[:, 1, :],
                        qk[CQ:128, j1 * 128:(j1 + 1) * 128],
                        kq[CQ:128, tsl],
                        start=True,
                        stop=True,
                        tile_position=(64, 0),
                    )
                    nc.scalar.activation(
                        ets[tb][:, j0:j0 + 2, :], sc[:, :, :], AF.Exp
                    )
                    if jp == 0:
                        esums[tb] = small.tile(
                            [128, TB], F32, tag="esum", name=f"esum_{tb}"
                        )
                        nc.vector.tensor_add(
                            esums[tb][:], ets[tb][:, 0, :], ets[tb][:, 1, :]
                        )
                    else:
                        nc.vector.tensor_add(
                            esums[tb][:], esums[tb][:], ets[tb][:, j0, :]
                        )
                        nc.vector.tensor_add(
                            esums[tb][:], esums[tb][:], ets[tb][:, j1, :]
                        )
                    if jp == NPAIR - 1:
                        # denominator: fold partitions on GPSIMD (result is
                        # broadcast to all rows), then fast reciprocal
                        dsum = small.tile(
                            [128, TB], F32, tag="dsum", name=f"dsum_{tb}"
                        )
                        nc.gpsimd.partition_all_reduce(
                            dsum[:], esums[tb][:], channels=128,
                            reduce_op=bass_isa.ReduceOp.add,
                        )
                        rbs[tb] = small.tile(
                            [128, TB], F32, tag="rb", name=f"rb_{tb}"
                        )
                        nc.vector.reciprocal_approx_fast(rbs[tb][:], dsum[:])

                def emit_consume_A(tb, jp):
                    for idx in (0, 1):
                        j = 2 * jp + idx
                        for ck in range(2):
                            nc.tensor.matmul(
                                avA[tb][ck][:],
                                vT[:, j, ck * 128:(ck + 1) * 128],
                                ets[tb][:, j, :],
                                start=(j == 0),
                                stop=(j == NSC - 1),
                            )

                def emit_consume_B(tb, jp):
                    if jp == 0:
                        avB[tb] = [
                            ps.tile([128, TB], F32, tag=f"avb{ck}",
                                    name=f"avB{ck}_{tb}")
                            for ck in range(2)
                        ]
                    for idx in (0, 1):
                        j = 2 * jp + idx
                        for ck in range(2):
                            nc.tensor.matmul(
                                avB[tb][ck][:],
                                vT[:, j, (2 + ck) * 128:(3 + ck) * 128],
                                ets[tb][:, j, :],
                                start=(j == 0),
                                stop=(j == NSC - 1),
                            )

                def finish(tb, half):
                    tsl = slice(tb * TB, (tb + 1) * TB)
                    avs = avA[tb] if half == 0 else avB[tb]
                    for i in range(2):
                        ck = 2 * half + i
                        ot = outp.tile(
                            [128, TB], F32, tag=f"ot{ck}", name=f"ot{ck}_{tb}"
                        )
                        nc.vector.tensor_mul(ot[:], avs[i][:], rbs[tb][:])
                        nc.scalar.dma_start(
                            out=out_d[ck * 128:(ck + 1) * 128, tsl], in_=ot[:]
                        )

                # ---- preamble: x chunks with projections + t-block 0 chasing
                pending = None
                start_block(0)
                for m in range(NTB):
                    msl = slice(m * TB, (m + 1) * TB)
                    nc.sync.dma_start(out=x_s[:, :, msl], in_=x_r[:, :, msl])
                    for th in range(2):
                        ths = slice(m * TB + th * 256, m * TB + (th + 1) * 256)
                        if th == 0:
                            nc.vector.tensor_copy(x_bf[:, :, ths], x_s[:, :, ths])
                        else:
                            nc.scalar.activation(
                                x_bf[:, :, ths], x_s[:, :, ths], AF.Copy
                            )

                    # packed Q/K projections, both layouts
                    qkp = ps.tile([128, 2, TB], F32, tag="sc", bufs=2,
                                  name=f"qkp_{m}")
                    for ci in range(NCH):
                        nc.tensor.matmul(
                            qkp[:, 0, :], wqkT[:, ci, :], x_bf[:, ci, msl],
                            start=(ci == 0), stop=(ci == NCH - 1),
                        )
                    for ci in range(NCH):
                        nc.tensor.matmul(
                            qkp[:, 1, :], wkqT[:, ci, :], x_bf[:, ci, msl],
                            start=(ci == 0), stop=(ci == NCH - 1),
                        )
                    nc.vector.tensor_scalar_add(qk[:, msl], qkp[:, 0, :], bqk[:, 0:1])
                    nc.vector.tensor_scalar_add(kq[:, msl], qkp[:, 1, :], bkq[:, 0:1])

                    # V^T projection for this chunk's four s-chunks
                    for j in range(4 * m, 4 * m + 4):
                        psv = ps.tile([128, C], F32, tag=f"avb{j % 2}",
                                      name=f"vp_{j}")
                        for ci in range(NCH):
                            nc.tensor.matmul(
                                psv[:],
                                x_bf[:, ci, j * 128:(j + 1) * 128],
                                wvT[:, ci, :],
                                start=(ci == 0),
                                stop=(ci == NCH - 1),
                            )
                        nc.vector.tensor_add(vT[:, j, :], psv[:], bv_bcast[:])

                    # t-block 0 scores/AV chase the preamble
                    for jp in (2 * m, 2 * m + 1):
                        emit_scores(0, jp)
                        if pending is not None:
                            emit_consume_A(*pending)
                        pending = (0, jp)

                # ---- main phases
                for tb in range(1, NTB):
                    start_block(tb)
                    for jp in range(NPAIR):
                        emit_scores(tb, jp)
                        ptb, pjp = pending
                        emit_consume_A(ptb, pjp)
                        if pjp == NPAIR - 1:
                            finish(ptb, 0)
                        pending = (tb, jp)
                        emit_consume_B(tb - 1, jp)
                        if jp == NPAIR - 1:
                            finish(tb - 1, 1)
                # tail: last block's A remainder and full B pass
                ptb, pjp = pending
                emit_consume_A(ptb, pjp)
                finish(ptb, 0)
                for jp in range(NPAIR):
                    emit_consume_B(NTB - 1, jp)
                finish(NTB - 1, 1)

    nc.compile()
    return nc


_PROGRAM = None


def _get_program() -> bass.Bass:
    global _PROGRAM
    if _PROGRAM is None:
        _PROGRAM = _build_program()
    return _PROGRAM


def kernel(**inputs: np.ndarray) -> np.ndarray:
    x = np.ascontiguousarray(np.asarray(inputs["x"], dtype=np.float32))
    wq = np.ascontiguousarray(np.asarray(inputs["Wq"], dtype=np.float32))
    bq = np.ascontiguousarray(np.asarray(inputs["bq"], dtype=np.float32)).reshape(CQ, 1)
    wk = np.ascontiguousarray(np.asarray(inputs["Wk"], dtype=np.float32))
    bk = np.ascontiguousarray(np.asarray(inputs["bk"], dtype=np.float32)).reshape(CQ, 1)
    wv = np.ascontiguousarray(np.asarray(inputs["Wv"], dtype=np.float32))
    bv = np.ascontiguousarray(np.asarray(inputs["bv"], dtype=np.float32)).reshape(1, C)

    nc = _get_program()
    in_maps = [
        {
            "x": np.ascontiguousarray(x[b]),
            "Wq": wq,
            "bq": bq,
            "Wk": wk,
            "bk": bk,
            "Wv": wv,
            "bv": bv,
        }
        for b in range(NCORES)
    ]
    res = run_bass_kernel_spmd(nc, in_maps, list(range(NCORES)))
    out = np.stack([res.results[b]["out"] for b in range(NCORES)], axis=0)
    return out.astype(np.float32)


if __name__ == "__main__":
    import reference

    inputs = {k: np.asarray(v) for k, v in reference.setup_inputs().items()}
    expected = np.asarray(reference.reference(**inputs))
    actual = kernel(**inputs)
    rel = np.linalg.norm(actual - expected) / np.linalg.norm(expected)
    print("Relative error:", rel)
